# revision 1
# baseline (speedup 1.0000x reference)
"""Trainium2 Bass kernel for nn_Attend: softmax(q@k^T * scale + bias) @ v.

Shapes (full problem):
  q:         [B=2, H=8, S=2048, D=64] fp32
  k, v:      [B=2, S=2048, D=64]      fp32 (shared across heads)
  mask:      [B=2, S=2048] bool       (all ones in practice)
  attn_bias: [B=2, H=8, S=2048, S=2048] fp32
  out:       [B=2, H=8, S=2048, D=64] fp32

Sharding: 16 (b,h) pairs over 8 cores -> 2 heads per core, k/v replicated
per-b (4 cores share each b).

Per-core algorithm (all layouts chosen so no large transposes of the
attention matrix are ever needed):
  - kT, qT built once via PE transposes ([64, S] layout, f32r, q pre-scaled).
  - S^T[j, i] computed per (head, 512-wide i-chunk, 128-wide j-tile):
      matmul(kT_tile, qT_chunk) into PSUM, then the bias tile is added by
      4x PE transpose-mode matmuls that ACCUMULATE into the same PSUM bank
      (start=False) -- bias is loaded in natural [i, j] layout (fast DMA)
      and transposed for free inside the accumulation.
  - P^T = exp(S^T) via ScalarE directly PSUM -> SBUF (bf16).
  - out^T[d, i] accumulated over j-tiles: matmul(v_aug, P^T) where v_aug has
    a ones-column appended -> row 64 of out^T is the softmax denominator.
  - Epilogue: small PE transposes back to [i, d], reciprocal + scale, DMA out.
"""

import sys

sys.path.insert(0, "/opt/trn_rl_repo")

from contextlib import ExitStack

import numpy as np

B, H, S, D = 2, 8, 2048, 64
NH = 2          # heads per core
N_CORES = 8
IC = S // 512   # i-chunks per head
JT = S // 128   # j-tiles

_cache = {}


def _build():
    import concourse.bacc as bacc
    import concourse.tile as tile
    from concourse import masks, mybir

    f32 = mybir.dt.float32
    f32r = mybir.dt.float32r
    bf16 = mybir.dt.bfloat16
    Exp = mybir.ActivationFunctionType.Exp

    nc = bacc.Bacc("TRN2", target_bir_lowering=False, debug=False,
                   num_devices=N_CORES)
    q_ap = nc.dram_tensor("q", [NH, S, D], f32, kind="ExternalInput").ap()
    k_ap = nc.dram_tensor("k", [S, D], f32, kind="ExternalInput").ap()
    v_ap = nc.dram_tensor("v", [S, D], f32, kind="ExternalInput").ap()
    bias_ap = nc.dram_tensor("bias", [NH, S, S], f32, kind="ExternalInput").ap()
    out_ap = nc.dram_tensor("out", [NH, S, D], f32, kind="ExternalOutput").ap()

    with tile.TileContext(nc) as tc, ExitStack() as ctx:
        const_pool = ctx.enter_context(tc.tile_pool(name="const", bufs=1))
        prep_sb = ctx.enter_context(tc.tile_pool(name="prep_sb", bufs=4))
        small_ps = ctx.enter_context(
            tc.tile_pool(name="small_ps", bufs=2, space="PSUM"))
        bias_pool = ctx.enter_context(tc.tile_pool(name="bias", bufs=2))
        st_pool = ctx.enter_context(
            tc.tile_pool(name="st", bufs=3, space="PSUM"))
        pt_pool = ctx.enter_context(tc.tile_pool(name="pt", bufs=4))
        ov_pool = ctx.enter_context(
            tc.tile_pool(name="ov", bufs=2, space="PSUM"))
        epi_sb = ctx.enter_context(tc.tile_pool(name="epi_sb", bufs=2))

        ident = const_pool.tile([128, 128], f32)
        masks.make_identity(nc, ident[:])

        # ---- prep: kT [64, S] f32r, qT [64, NH*S] f32r (scaled), v_aug bf16
        kT = const_pool.tile([64, S], f32r)
        qT = const_pool.tile([64, NH * S], f32r)
        v_aug = const_pool.tile([128, JT * 65], bf16)
        nc.vector.memset(v_aug[:], 1.0)
        for jt in range(JT):
            t = prep_sb.tile([128, 64], f32, tag="prep")
            nc.sync.dma_start(t[:], k_ap[jt * 128:(jt + 1) * 128, :])
            p = small_ps.tile([64, 128], f32, tag="sm")
            nc.tensor.matmul(p[:], t[:], ident[:], is_transpose=True,
                             start=True, stop=True)
            nc.vector.tensor_copy(kT[:, jt * 128:(jt + 1) * 128], p[:])
            tv = prep_sb.tile([128, 64], f32, tag="prep")
            nc.sync.dma_start(tv[:], v_ap[jt * 128:(jt + 1) * 128, :])
            nc.vector.tensor_copy(v_aug[:, jt * 65:jt * 65 + 64], tv[:])
        for h in range(NH):
            for jt in range(JT):
                t = prep_sb.tile([128, 64], f32, tag="prep")
                nc.sync.dma_start(t[:], q_ap[h, jt * 128:(jt + 1) * 128, :])
                p = small_ps.tile([64, 128], f32, tag="sm")
                nc.tensor.matmul(p[:], t[:], ident[:], is_transpose=True,
                                 start=True, stop=True)
                nc.vector.tensor_scalar_mul(
                    qT[:, h * S + jt * 128: h * S + (jt + 1) * 128], p[:],
                    float(D) ** -0.5)

        # ---- main: per (head, i-chunk of 512)
        for h in range(NH):
            for c in range(IC):
                bias_t = bias_pool.tile([128, 4, S], f32)
                nc.sync.dma_start(
                    bias_t[:],
                    bias_ap[h, c * 512:(c + 1) * 512, :].rearrange(
                        "(s p) j -> p s j", p=128))
                ov = ov_pool.tile([65, 512], f32)
                for jt in range(JT):
                    st = st_pool.tile([128, 512], f32)
                    nc.tensor.matmul(
                        st[:], kT[:, jt * 128:(jt + 1) * 128],
                        qT[:, h * S + c * 512: h * S + (c + 1) * 512],
                        start=True, stop=False)
                    for s in range(4):
                        nc.tensor.matmul(
                            st[:, s * 128:(s + 1) * 128],
                            bias_t[:, s, jt * 128:(jt + 1) * 128], ident[:],
                            is_transpose=True, start=False, stop=(s == 3),
                            skip_group_check=True)
                    pt = pt_pool.tile([128, 512], bf16)
                    nc.scalar.activation(pt[:], st[:], Exp)
                    nc.tensor.matmul(
                        ov[:], v_aug[:, jt * 65: jt * 65 + 65], pt[:],
                        start=(jt == 0), stop=(jt == JT - 1),
                        skip_group_check=True)
                # epilogue: out rows = ov[:64, :] / ov[64, :]
                ovs = epi_sb.tile([65, 512], f32, tag="ovs")
                nc.vector.tensor_copy(ovs[:], ov[:])
                res = epi_sb.tile([128, 4, 64], f32, tag="res")
                for s in range(4):
                    tp = small_ps.tile([128, 65], f32, tag="sm")
                    nc.tensor.matmul(tp[:], ovs[:, s * 128:(s + 1) * 128],
                                     ident[:65, :65], is_transpose=True,
                                     start=True, stop=True)
                    rec = epi_sb.tile([128, 1], f32, tag="rec")
                    nc.vector.reciprocal(rec[:], tp[:, 64:65])
                    nc.vector.tensor_scalar_mul(res[:, s, :], tp[:, 0:64],
                                                rec[:])
                nc.sync.dma_start(
                    out_ap[h, c * 512:(c + 1) * 512, :].rearrange(
                        "(s p) d -> p s d", p=128), res[:])

    nc.compile()
    return nc


def kernel(q, k, v, mask, attn_bias):
    from concourse.bass_utils import run_bass_kernel_spmd

    q = np.ascontiguousarray(np.asarray(q, dtype=np.float32))
    k = np.ascontiguousarray(np.asarray(k, dtype=np.float32))
    v = np.ascontiguousarray(np.asarray(v, dtype=np.float32))
    mask = np.asarray(mask)
    attn_bias = np.asarray(attn_bias, dtype=np.float32)

    if not mask.all():
        attn_bias = np.where(mask[:, None, None, :], attn_bias,
                             np.float32(-3.0e38)).astype(np.float32)

    if "nc" not in _cache:
        _cache["nc"] = _build()
    nc = _cache["nc"]

    in_maps = []
    for c in range(N_CORES):
        b = c // 4
        h0 = NH * (c % 4)
        in_maps.append({
            "q": np.ascontiguousarray(q[b, h0:h0 + NH]),
            "k": k[b],
            "v": v[b],
            "bias": np.ascontiguousarray(attn_bias[b, h0:h0 + NH]),
        })
    res = run_bass_kernel_spmd(nc, in_maps, core_ids=list(range(N_CORES)))
    out = np.empty((B, H, S, D), dtype=np.float32)
    for c in range(N_CORES):
        b = c // 4
        h0 = NH * (c % 4)
        out[b, h0:h0 + NH] = res.results[c]["out"]
    return out
